# revision 2
# baseline (speedup 1.0000x reference)
"""Trainium2 Bass kernel v3 for DynConv2d (DGCNN-style edge conv).

Algebraic reduction (unchanged from v2):
  out[n, c] = u_n[c] + max_{j in top16(n)} v_j[c]
  u = (W1 - W2) @ feats + bias             # [128, r]
  v = W2 @ feats                           # [128, n]
  key[i, j] = <f_i, f_j> - 0.5*|f_j|^2     # row-wise top-16 ordering

v3 changes vs v2 (baseline 824us/iter device, DVE 94% busy):
1. Index-packed keys kill both full-row max_index passes (2 x 8.6us/tile of
   DVE). The column index rides in the low 16 bits of each key word:
   kp[m, j] = fp16(key[m, j])<<16 | j. The fp16 halves are written by the
   ScalarE PSUM->SBUF cast (strided, into the high halves of a u32 buffer
   whose low halves hold a persistent iota), so packing costs no extra pass.
   For positive/negative finite fp16 keys the packed word compares correctly
   as fp32 (bit-monotone), so max8/match_replace yield top-16 values AND
   indices together; extraction is a [128,16] bitwise AND.
   Accuracy: fp16 key quantization flips near-tied neighbors only;
   measured end-to-end rel err ~6e-3 (tolerance 2e-2).
2. Keys matmul in float32r (1 cycle/row vs 4 for fp32): PE per tile drops
   13.7us -> ~3.6us. Measured key precision 1.1e-4 rel (vs 4e-8 fp32) --
   ordering-irrelevant.
3. v / u matmuls with fp16 inputs (1 cycle/row), fp32 PSUM accumulate.
4. dma_gather of fp16 v rows unchanged from v2 (16 SDMA engines, 2x1024
   idxs/tile, ~7us/tile).
"""

import sys

for _p in ("/opt/trn_rl_repo", "/root/.axon_site/_ro/trn_rl_repo"):
    if _p not in sys.path:
        sys.path.insert(0, _p)

import numpy as np

B = 4
CIN = 64
COUT = 128
N = 8192
K = 16
N_CORES = 8

_prog_cache = {}


def build_program(n=N, r=N // 2, num_devices=N_CORES, repeat=1,
                  no_topk=False, no_gather=False, minimal=False,
                  no_reduce=False, gp_reduce=False, direct_f32r=True):
    import concourse.bacc as bacc
    import concourse.mybir as mybir
    import concourse.tile as tile

    f32 = mybir.dt.float32
    f32r = mybir.dt.float32r
    f16 = mybir.dt.float16
    i16 = mybir.dt.int16
    i32 = mybir.dt.int32
    u32 = mybir.dt.uint32
    CH = 512                 # keys matmul chunk (PSUM bank)
    nch = n // CH            # 16
    CK = 256                 # L1 top-8 chunk
    nck = n // CK            # 32
    NB = n // 128            # v blocks
    rt_count = r // 128
    NIDX = 128 * K           # 2048 gathered columns per row tile

    nc = bacc.Bacc("TRN2", target_bir_lowering=False, debug=False,
                   num_devices=num_devices)

    featsr_d = nc.dram_tensor("featsr", [CIN + 2, n], f32r, kind="ExternalInput")
    featslr_d = nc.dram_tensor("featslr", [CIN + 2, r], f32r, kind="ExternalInput")
    feats16_d = nc.dram_tensor("feats16", [CIN, n], f16, kind="ExternalInput")
    featsl16_d = nc.dram_tensor("featsl16", [CIN, r], f16, kind="ExternalInput")
    w2t16_d = nc.dram_tensor("w2t16", [CIN, COUT], f16, kind="ExternalInput")
    wdt16_d = nc.dram_tensor("wdt16", [CIN, COUT], f16, kind="ExternalInput")
    bias_d = nc.dram_tensor("bias", [COUT, 1], f32, kind="ExternalInput")
    ident_d = nc.dram_tensor("ident", [128, 128], f32, kind="ExternalInput")
    iota_d = nc.dram_tensor("iota", [128, n], u32, kind="ExternalInput")
    out_d = nc.dram_tensor("out", [COUT, r], f32, kind="ExternalOutput")

    with tile.TileContext(nc) as tc:
        with tc.tile_pool(name="const", bufs=1) as const, \
             tc.tile_pool(name="kpp", bufs=2) as kpp, \
             tc.tile_pool(name="vg", bufs=4) as vgp, \
             tc.tile_pool(name="small", bufs=4) as small, \
             tc.tile_pool(name="dram", bufs=1, space="DRAM") as dramp, \
             tc.tile_pool(name="psk", bufs=4, space="PSUM") as psk, \
             tc.tile_pool(name="psa", bufs=2, space="PSUM") as psa:

            # ---------------- prologue ----------------
            featsr = const.tile([CIN + 2, n], f32r)
            featslr = const.tile([CIN + 2, r], f32r)
            nc.sync.dma_start(featsr[:, :], featsr_d.ap())
            nc.sync.dma_start(featslr[:, :], featslr_d.ap())

            feats16 = const.tile([CIN, n], f16)
            featsl16 = const.tile([CIN, r], f16)
            nc.sync.dma_start(feats16[:, :], feats16_d.ap())
            nc.sync.dma_start(featsl16[:, :], featsl16_d.ap())

            w2t16 = const.tile([CIN, COUT], f16)
            nc.sync.dma_start(w2t16[:, :], w2t16_d.ap())
            wdt16 = const.tile([CIN, COUT], f16)
            nc.sync.dma_start(wdt16[:, :], wdt16_d.ap())
            bias = const.tile([COUT, 1], f32)
            nc.sync.dma_start(bias[:, :], bias_d.ap())
            ident = const.tile([128, 128], f32)
            nc.sync.dma_start(ident[:, :], ident_d.ap())

            ut = const.tile([COUT, r], f32)
            vt_dram = dramp.tile([n, 128], f16)

            # iota into both kp rotation buffers (low halves persist; the
            # per-tile ScalarE cast only rewrites the fp16 high halves)
            kp_bufs = []
            for _ in range(2):
                kp = kpp.tile([128, n], u32, tag="kp")
                nc.sync.dma_start(kp[:, :], iota_d.ap())
                kp_bufs.append(kp)

            # vT staging: vt_dram[j, c] = v[c, j] (fp16), per 128-row block
            for bb in range(NB):
                bsl = slice(bb * 128, (bb + 1) * 128)
                pv = psa.tile([128, COUT], f32, tag="psa")
                nc.tensor.matmul(pv[:, :], feats16[:, bsl], w2t16[:, :],
                                 start=True, stop=True)
                vstage = small.tile([128, COUT], f16, tag="vstage")
                nc.scalar.copy(vstage[:, :], pv[:, :])
                nc.sync.dma_start(vt_dram[bsl, :], vstage[:, :])

            # u = (W1-W2) @ featsl + bias  -> [128, r]
            for c in range(r // CH):
                sl = slice(c * CH, (c + 1) * CH)
                pu = psa.tile([COUT, CH], f32, tag="psa")
                nc.tensor.matmul(pu[:, :], wdt16[:, :], featsl16[:, sl],
                                 start=True, stop=True)
                nc.vector.tensor_scalar_add(ut[:, sl], pu[:, :], bias[:, :])

            # ---------------- main loop over row tiles ----------------
            def stage_a(rt):
                rsl = slice(rt * 128, (rt + 1) * 128)
                kp = kpp.tile([128, n], u32, tag="kp")
                kp16 = kp.bitcast(f16).rearrange("p (n two) -> p n two", two=2)
                kpf = kp.bitcast(f32)
                for c in range(nch):
                    sl = slice(c * CH, (c + 1) * CH)
                    pk = psk.tile([128, CH], f32, tag="psk")
                    nc.tensor.matmul(pk[:, :], featslr[:, rsl], featsr[:, sl],
                                     start=True, stop=True)
                    nc.scalar.activation(kp16[:, sl, 1], pk[:, :],
                                         mybir.ActivationFunctionType.Copy)

                if minimal:
                    ot0 = small.tile([128, 128], f32, tag="ot")
                    nc.vector.tensor_add(ot0[:, :], kpf[:, 0:128], ut[:, rsl])
                    nc.sync.dma_start(out_d.ap()[:, rsl], ot0[:, :])
                    return None

                jff = small.tile([128, 16], f32, tag="jff")
                if no_topk:
                    nc.vector.memset(jff[:, :], 5.0)
                else:
                    # L1: top-8 packed of each 256-col chunk -> 256 candidates
                    l1val = small.tile([128, 8 * nck], f32, tag="l1")
                    for c in range(nck):
                        nc.vector.max(l1val[:, 8 * c:8 * (c + 1)],
                                      kpf[:, CK * c:CK * (c + 1)])
                    # L2: top-16 packed of the candidates
                    r12 = small.tile([128, 16], f32, tag="r12")
                    nc.vector.max(r12[:, 0:8], l1val[:, :])
                    nc.vector.match_replace(l1val[:, :], r12[:, 0:8],
                                            l1val[:, :], -3.0e38)
                    nc.vector.max(r12[:, 8:16], l1val[:, :])
                    # extract the column index from the low 16 bits
                    jfi = small.tile([128, 16], i32, tag="jfi")
                    nc.vector.tensor_scalar(jfi[:, :], r12[:, :].bitcast(i32),
                                            0xFFFF, None,
                                            mybir.AluOpType.bitwise_and)
                    nc.vector.tensor_copy(jff[:, :], jfi[:, :])
                return jff

            def stage_b(rt, jff):
                rsl = slice(rt * 128, (rt + 1) * 128)
                # wrapped int16 index layout: widx[16g + q, m] = j[m, q]
                tp = psa.tile([16, 128], f32, tag="tp")
                nc.tensor.transpose(tp[:, :], jff[:, :], ident[:, :])
                tpi = small.tile([16, 128], i16, tag="tpi")
                nc.scalar.copy(tpi[:, :], tp[:, :])
                widx = small.tile([128, 128], i16, tag="widx")
                for g in range(8):
                    nc.sync.dma_start(widx[16 * g:16 * (g + 1), :], tpi[:, :])

                mx = small.tile([128, 128], f32, tag="mx")
                if no_gather:
                    nc.vector.tensor_copy(mx[:, :], ut[:, rsl])
                else:
                    vg = vgp.tile([128, NIDX], f16, tag="vg")
                    for s in range(NIDX // 1024):
                        nc.gpsimd.dma_gather(
                            vg[:, 1024 * s:1024 * (s + 1)]
                            .rearrange("p (o i) -> p o i", o=1),
                            vt_dram[:, :], widx[:, 64 * s:64 * (s + 1)],
                            num_idxs=1024, num_idxs_reg=1024,
                            elem_size=128, transpose=True,
                            single_packet=False)
                    if no_reduce:
                        nc.vector.tensor_copy(mx[:, :], vg[:, 0:128])
                    else:
                        eng = nc.gpsimd if gp_reduce else nc.vector
                        eng.reduce_max(mx[:, :],
                                       vg[:, :].rearrange("p (g k) -> p g k",
                                                          k=K),
                                       axis=mybir.AxisListType.X)
                ot = small.tile([128, 128], f32, tag="ot")
                nc.vector.tensor_add(ot[:, :], mx[:, :], ut[:, rsl])
                nc.sync.dma_start(out_d.ap()[:, rsl], ot[:, :])

            def main_body():
                DEPTH = 2
                pending = []
                for rt in range(rt_count):
                    jff = stage_a(rt)
                    if minimal:
                        continue
                    pending.append((rt, jff))
                    if len(pending) > DEPTH:
                        prt, pjff = pending.pop(0)
                        stage_b(prt, pjff)
                for prt, pjff in pending:
                    stage_b(prt, pjff)

            if repeat > 1:
                with tc.For_i(0, repeat, 1):
                    main_body()
            else:
                main_body()

    nc.compile()
    return nc


def _get_program(n, r, num_devices):
    key = (n, r, num_devices)
    if key not in _prog_cache:
        _prog_cache[key] = build_program(n, r, num_devices)
    return _prog_cache[key]


def make_in_maps(xb, W, b, n, r, num):
    """xb: [B, 64, n] f32. Returns per-core input dicts."""
    W1 = W[:, :CIN]
    W2 = W[:, CIN:]
    w2t16 = np.ascontiguousarray(W2.T).astype(np.float16)
    wdt16 = np.ascontiguousarray((W1 - W2).T).astype(np.float16)
    bias = b.reshape(COUT, 1).astype(np.float32)
    ident = np.eye(128, dtype=np.float32)
    iota = np.broadcast_to(np.arange(n, dtype=np.uint32)[None, :],
                           (128, n)).copy()
    in_maps = []
    for core in range(num):
        bi, half = core // 2, core % 2
        f = xb[bi]                                    # [64, n] f32
        xx = (f * f).sum(0)
        featsr = np.concatenate([
            f,
            (-0.5 * xx)[None, :],
            np.ones((1, n), np.float32),
        ], 0).astype(np.float32)
        fl = f[:, half * r:(half + 1) * r]
        featslr = np.concatenate([
            fl,
            np.ones((1, r), np.float32),
            np.zeros((1, r), np.float32),             # per-row key bias
        ], 0).astype(np.float32)
        in_maps.append({
            "featsr": featsr,
            "featslr": featslr,
            "feats16": f.astype(np.float16),
            "featsl16": fl.astype(np.float16),
            "w2t16": w2t16, "wdt16": wdt16, "bias": bias,
            "ident": ident, "iota": iota,
        })
    return in_maps


def run_cores(xb, W, b, n, r, trace=False):
    from concourse.bass_utils import run_bass_kernel_spmd

    num = N_CORES
    in_maps = make_in_maps(xb, W, b, n, r, num)
    nc = _get_program(n, r, num)
    res = run_bass_kernel_spmd(nc, in_maps, core_ids=list(range(num)),
                               trace=trace)
    return [res.results[i]["out"] for i in range(num)], res


def kernel(x, W, b):
    """Full-input entry point: x [4, 64, 8192, 1] f32 -> [4, 128, 8192, 1]."""
    x = np.asarray(x, dtype=np.float32)
    W = np.asarray(W, dtype=np.float32)
    b = np.asarray(b, dtype=np.float32)
    xb = np.ascontiguousarray(x[:, :, :, 0])
    r = N // 2
    outs, _ = run_cores(xb, W, b, N, r)
    out = np.empty((B, COUT, N, 1), np.float32)
    for core in range(N_CORES):
        bi, half = core // 2, core % 2
        out[bi, :, half * r:(half + 1) * r, 0] = outs[core]
    return out


# revision 18
# speedup vs baseline: 5.2557x; 5.2557x over previous
"""Trainium2 Bass kernel v3 for DynConv2d (DGCNN-style edge conv).

Algebraic reduction (unchanged from v2):
  out[n, c] = u_n[c] + max_{j in top16(n)} v_j[c]
  u = (W1 - W2) @ feats + bias             # [128, r]
  v = W2 @ feats                           # [128, n]
  key[i, j] = <f_i, f_j> - 0.5*|f_j|^2     # row-wise top-16 ordering

v3 changes vs v2 (baseline 824us/iter device, DVE 94% busy):
1. Index-packed keys kill both full-row max_index passes (2 x 8.6us/tile of
   DVE). The column index rides in the low 16 bits of each key word:
   kp[m, j] = fp16(key[m, j])<<16 | j. The fp16 halves are written by the
   ScalarE PSUM->SBUF cast (strided, into the high halves of a u32 buffer
   whose low halves hold a persistent iota), so packing costs no extra pass.
   For positive/negative finite fp16 keys the packed word compares correctly
   as fp32 (bit-monotone), so max8/match_replace yield top-16 values AND
   indices together; extraction is a [128,16] bitwise AND.
   Accuracy: fp16 key quantization flips near-tied neighbors only;
   measured end-to-end rel err ~6e-3 (tolerance 2e-2).
2. Keys matmul in float32r (1 cycle/row vs 4 for fp32): PE per tile drops
   13.7us -> ~3.6us. Measured key precision 1.1e-4 rel (vs 4e-8 fp32) --
   ordering-irrelevant.
3. v / u matmuls with fp16 inputs (1 cycle/row), fp32 PSUM accumulate.
4. dma_gather of fp16 v rows unchanged from v2 (16 SDMA engines, 2x1024
   idxs/tile, ~7us/tile).
"""

import sys

for _p in ("/opt/trn_rl_repo", "/root/.axon_site/_ro/trn_rl_repo"):
    if _p not in sys.path:
        sys.path.insert(0, _p)

import numpy as np

B = 4
CIN = 64
COUT = 128
N = 8192
K = 16
N_CORES = 8

_prog_cache = {}


def build_program(n=N, r=N // 2, num_devices=N_CORES, repeat=1,
                  no_topk=False, no_gather=False, minimal=False,
                  no_reduce=False, mm_dtype='f16', cast_mode='strided',
                  depth=3, act_wide=True, gp_tree=False, gp_misc=False,
                  ck=512):
    import concourse.bacc as bacc
    import concourse.mybir as mybir
    import concourse.tile as tile

    f32 = mybir.dt.float32
    f32r = mybir.dt.float32r
    f16 = mybir.dt.float16
    i16 = mybir.dt.int16
    i32 = mybir.dt.int32
    u32 = mybir.dt.uint32
    CH = 512                 # keys matmul chunk (PSUM bank)
    nch = n // CH            # 16
    CK = ck                  # L1 top-8 chunk
    nck = n // CK            # 32
    NB = n // 128            # v blocks
    rt_count = r // 128
    NIDX = 128 * K           # 2048 gathered columns per row tile

    nc = bacc.Bacc("TRN2", target_bir_lowering=False, debug=False,
                   num_devices=num_devices)

    if mm_dtype == 'split16':
        fh_d = nc.dram_tensor("feats_hi", [CIN + 2, n], f16, kind="ExternalInput")
        flo_d = nc.dram_tensor("feats_lo", [CIN + 2, n], f16, kind="ExternalInput")
        flh_d = nc.dram_tensor("featsl_hi", [CIN + 2, r], f16, kind="ExternalInput")
        fll_d = nc.dram_tensor("featsl_lo", [CIN + 2, r], f16, kind="ExternalInput")
    else:
        kdt = {'f32r': f32r, 'f32': f32, 'f16': f16}[mm_dtype]
        featsr_d = nc.dram_tensor("featsr", [CIN + 2, n], kdt, kind="ExternalInput")
        featslr_d = nc.dram_tensor("featslr", [CIN + 2, r], kdt, kind="ExternalInput")
    feats16_d = nc.dram_tensor("feats16", [CIN, n], f16, kind="ExternalInput")
    featsl16_d = nc.dram_tensor("featsl16", [CIN, r], f16, kind="ExternalInput")
    w2t16_d = nc.dram_tensor("w2t16", [CIN, COUT], f16, kind="ExternalInput")
    wdt16_d = nc.dram_tensor("wdt16", [CIN, COUT], f16, kind="ExternalInput")
    bias_d = nc.dram_tensor("bias", [COUT, 1], f32, kind="ExternalInput")
    ident_d = nc.dram_tensor("ident", [128, 128], f32, kind="ExternalInput")
    out_d = nc.dram_tensor("out", [COUT, r], f32, kind="ExternalOutput")

    with tile.TileContext(nc) as tc:
        with tc.tile_pool(name="const", bufs=1) as const, \
             tc.tile_pool(name="kpp", bufs=2) as kpp, \
             tc.tile_pool(name="vg", bufs=4) as vgp, \
             tc.tile_pool(name="small", bufs=4) as small, \
             tc.tile_pool(name="dram", bufs=1, space="DRAM") as dramp, \
             tc.tile_pool(name="psk", bufs=(3 if act_wide else 6), space="PSUM") as psk, \
             tc.tile_pool(name="psa", bufs=1, space="PSUM") as psa:

            # ---------------- prologue ----------------
            if mm_dtype == 'split16':
                fh = const.tile([CIN + 2, n], f16)
                flo = const.tile([CIN + 2, n], f16)
                flh = const.tile([CIN + 2, r], f16)
                fll = const.tile([CIN + 2, r], f16)
                nc.sync.dma_start(fh[:, :], fh_d.ap())
                nc.sync.dma_start(flo[:, :], flo_d.ap())
                nc.sync.dma_start(flh[:, :], flh_d.ap())
                nc.sync.dma_start(fll[:, :], fll_d.ap())
            else:
                featsr = const.tile([CIN + 2, n], kdt)
                featslr = const.tile([CIN + 2, r], kdt)
                nc.sync.dma_start(featsr[:, :], featsr_d.ap())
                nc.sync.dma_start(featslr[:, :], featslr_d.ap())

            feats16 = const.tile([CIN, n], f16)
            featsl16 = const.tile([CIN, r], f16)
            nc.sync.dma_start(feats16[:, :], feats16_d.ap())
            nc.sync.dma_start(featsl16[:, :], featsl16_d.ap())

            w2t16 = const.tile([CIN, COUT], f16)
            nc.sync.dma_start(w2t16[:, :], w2t16_d.ap())
            wdt16 = const.tile([CIN, COUT], f16)
            nc.sync.dma_start(wdt16[:, :], wdt16_d.ap())
            bias = const.tile([COUT, 1], f32)
            nc.sync.dma_start(bias[:, :], bias_d.ap())
            ident = const.tile([128, 128], f32)
            nc.sync.dma_start(ident[:, :], ident_d.ap())

            ut = const.tile([COUT, r], f32)
            vt_dram = dramp.tile([n, 128], f16)

            # iota into both kp rotation buffers (low halves persist; the
            # per-tile ScalarE cast only rewrites the fp16 high halves)
            for _ in range(2):
                kp = kpp.tile([128, n], u32, tag="kp")
                nc.gpsimd.iota(kp[:, :], [[1, n]], channel_multiplier=0)

            # vT staging: vt_dram[j, c] = v[c, j] (fp16), per 128-row block
            for bb in range(NB):
                bsl = slice(bb * 128, (bb + 1) * 128)
                pv = psa.tile([128, COUT], f32, tag="psa")
                nc.tensor.matmul(pv[:, :], feats16[:, bsl], w2t16[:, :],
                                 start=True, stop=True)
                vstage = small.tile([128, COUT], f16, tag="vstage")
                nc.scalar.copy(vstage[:, :], pv[:, :])
                nc.sync.dma_start(vt_dram[bsl, :], vstage[:, :])

            # u = (W1-W2) @ featsl + bias  -> [128, r]
            for c in range(r // CH):
                sl = slice(c * CH, (c + 1) * CH)
                pu = psa.tile([COUT, CH], f32, tag="psa")
                nc.tensor.matmul(pu[:, :], wdt16[:, :], featsl16[:, sl],
                                 start=True, stop=True)
                nc.vector.tensor_scalar_add(ut[:, sl], pu[:, :], bias[:, :])

            # ---------------- main loop over row tiles ----------------
            def cast_chunk(kp, kp16, c, pk):
                sl = slice(c * CH, (c + 1) * CH)
                if cast_mode == 'strided':
                    nc.scalar.activation(kp16[:, sl, 1], pk[:, :],
                                         mybir.ActivationFunctionType.Copy)
                else:  # contig (timing bisect only; breaks packing semantics)
                    nc.scalar.activation(kp.bitcast(f16)[:, sl], pk[:, :],
                                         mybir.ActivationFunctionType.Copy)

            def stage_a(rt):
                rsl = slice(rt * 128, (rt + 1) * 128)
                kp = kpp.tile([128, n], u32, tag="kp")
                kp16 = kp.bitcast(f16).rearrange("p (n two) -> p n two", two=2)
                kpf = kp.bitcast(f32)
                if mm_dtype == 'split16':
                    # grouped over psk bufs to amortize fp16 ldweights:
                    # per group, 3 stationary loads serve 4 chunks each
                    GB = 4
                    for g in range(nch // GB):
                        cs = range(g * GB, (g + 1) * GB)
                        pks = {}
                        for c in cs:
                            pk_g = psk.tile([128, CH], f32, tag="psk", name=f"pk{c}")
                            pks[c] = pk_g
                        for c in cs:
                            sl = slice(c * CH, (c + 1) * CH)
                            nc.tensor.matmul(pks[c][:, :], flh[:, rsl],
                                             fh[:, sl], start=True, stop=False)
                        for c in cs:
                            sl = slice(c * CH, (c + 1) * CH)
                            nc.tensor.matmul(pks[c][:, :], flh[:, rsl],
                                             flo[:, sl], start=False, stop=False)
                        for c in cs:
                            sl = slice(c * CH, (c + 1) * CH)
                            nc.tensor.matmul(pks[c][:, :], fll[:, rsl],
                                             fh[:, sl], start=False, stop=True)
                        for c in cs:
                            cast_chunk(kp, kp16, c, pks[c])
                elif act_wide:
                    for c2 in range(nch // 2):
                        pk = psk.tile([128, 2 * CH], f32, tag="psk")
                        for h in range(2):
                            c = 2 * c2 + h
                            sl = slice(c * CH, (c + 1) * CH)
                            nc.tensor.matmul(pk[:, h * CH:(h + 1) * CH],
                                             featslr[:, rsl], featsr[:, sl],
                                             start=True, stop=True)
                        sl2 = slice(c2 * 2 * CH, (c2 + 1) * 2 * CH)
                        if cast_mode == 'strided':
                            nc.scalar.activation(kp16[:, sl2, 1], pk[:, :],
                                                 mybir.ActivationFunctionType.Copy)
                        else:
                            nc.scalar.activation(kp.bitcast(f16)[:, sl2], pk[:, :],
                                                 mybir.ActivationFunctionType.Copy)
                else:
                    for c in range(nch):
                        sl = slice(c * CH, (c + 1) * CH)
                        pk = psk.tile([128, CH], f32, tag="psk")
                        nc.tensor.matmul(pk[:, :], featslr[:, rsl],
                                         featsr[:, sl], start=True, stop=True)
                        cast_chunk(kp, kp16, c, pk)

                if minimal:
                    ot0 = small.tile([128, 128], f32, tag="ot")
                    nc.vector.tensor_add(ot0[:, :], kpf[:, 0:128], ut[:, rsl])
                    nc.sync.dma_start(out_d.ap()[:, rsl], ot0[:, :])
                    return None

                jff = small.tile([128, 16], f32, tag="jff")
                if no_topk:
                    nc.vector.memset(jff[:, :], 5.0)
                else:
                    # L1: top-8 packed of each 256-col chunk -> 256 candidates
                    l1val = small.tile([128, 8 * nck], f32, tag="l1")
                    for c in range(nck):
                        nc.vector.max(l1val[:, 8 * c:8 * (c + 1)],
                                      kpf[:, CK * c:CK * (c + 1)])
                    # L2: top-16 packed of the candidates
                    r12 = small.tile([128, 16], f32, tag="r12")
                    nc.vector.max(r12[:, 0:8], l1val[:, :])
                    nc.vector.match_replace(l1val[:, :], r12[:, 0:8],
                                            l1val[:, :], -3.0e38)
                    nc.vector.max(r12[:, 8:16], l1val[:, :])
                    # extract the column index from the low 16 bits
                    eng_x = nc.gpsimd if gp_misc else nc.vector
                    jfi = small.tile([128, 16], i32, tag="jfi")
                    eng_x.tensor_scalar(jfi[:, :], r12[:, :].bitcast(i32),
                                        0xFFFF, None,
                                        mybir.AluOpType.bitwise_and)
                    eng_x.tensor_copy(jff[:, :], jfi[:, :])
                return jff

            def stage_b(rt, jff):
                rsl = slice(rt * 128, (rt + 1) * 128)
                # wrapped int16 index layout: widx[16g + q, m] = j[m, q]
                tp = psa.tile([16, 128], f32, tag="tp")
                nc.tensor.transpose(tp[:, :], jff[:, :], ident[:, :])
                tpi = small.tile([16, 128], i16, tag="tpi")
                nc.scalar.copy(tpi[:, :], tp[:, :])
                widx = small.tile([128, 128], i16, tag="widx")
                for g in range(8):
                    nc.sync.dma_start(widx[16 * g:16 * (g + 1), :], tpi[:, :])

                mx = small.tile([128, 128], f32, tag="mx")
                if no_gather:
                    nc.vector.tensor_copy(mx[:, :], ut[:, rsl])
                else:
                    vg = vgp.tile([128, NIDX], f16, tag="vg")
                    for s in range(NIDX // 1024):
                        nc.gpsimd.dma_gather(
                            vg[:, 1024 * s:1024 * (s + 1)]
                            .rearrange("p (o i) -> p o i", o=1),
                            vt_dram[:, :], widx[:, 64 * s:64 * (s + 1)],
                            num_idxs=1024, num_idxs_reg=1024,
                            elem_size=128, transpose=True,
                            single_packet=False)
                    if no_reduce:
                        nc.vector.tensor_copy(mx[:, :], vg[:, 0:128])
                    elif gp_tree:
                        # grouped max via elementwise-max tree on gpsimd
                        vgv = vg[:, :]
                        w = NIDX // 2
                        while w >= 128:
                            a = vgv.rearrange("p (g k) -> p g k", k=2 * w // 128)
                            nc.gpsimd.tensor_tensor(
                                a[:, :, 0:w // 128], a[:, :, 0:w // 128],
                                a[:, :, w // 128:], mybir.AluOpType.max)
                            w //= 2
                        nc.gpsimd.tensor_copy(
                            mx[:, :],
                            vg[:, :].rearrange("p (g k) -> p g k", k=K)[:, :, 0])
                    else:
                        nc.vector.reduce_max(mx[:, :],
                                             vg[:, :]
                                             .rearrange("p (g k) -> p g k",
                                                        k=K),
                                             axis=mybir.AxisListType.X)
                ot = small.tile([128, 128], f32, tag="ot")
                eng_add = nc.gpsimd if gp_misc else nc.vector
                eng_add.tensor_add(ot[:, :], mx[:, :], ut[:, rsl])
                nc.sync.dma_start(out_d.ap()[:, rsl], ot[:, :])

            def main_body():
                DEPTH = depth
                pending = []
                for rt in range(rt_count):
                    jff = stage_a(rt)
                    if minimal:
                        continue
                    pending.append((rt, jff))
                    if len(pending) > DEPTH:
                        prt, pjff = pending.pop(0)
                        stage_b(prt, pjff)
                for prt, pjff in pending:
                    stage_b(prt, pjff)

            if repeat > 1:
                with tc.For_i(0, repeat, 1):
                    main_body()
            else:
                main_body()

    nc.compile()
    return nc


def _get_program(n, r, num_devices):
    key = (n, r, num_devices)
    if key not in _prog_cache:
        _prog_cache[key] = build_program(n, r, num_devices)
    return _prog_cache[key]


def make_in_maps(xb, W, b, n, r, num, split16=False, kdt=np.float16):
    """xb: [B, 64, n] f32. Returns per-core input dicts."""
    W1 = W[:, :CIN]
    W2 = W[:, CIN:]
    w2t16 = np.ascontiguousarray(W2.T).astype(np.float16)
    wdt16 = np.ascontiguousarray((W1 - W2).T).astype(np.float16)
    bias = b.reshape(COUT, 1).astype(np.float32)
    ident = np.eye(128, dtype=np.float32)
    in_maps = []
    for core in range(num):
        bi, half = core // 2, core % 2
        f = xb[bi]                                    # [64, n] f32
        xx = (f * f).sum(0)
        featsr = np.concatenate([
            f,
            (-0.5 * xx)[None, :],
            np.ones((1, n), np.float32),
        ], 0).astype(np.float32)
        fl = f[:, half * r:(half + 1) * r]
        featslr = np.concatenate([
            fl,
            np.ones((1, r), np.float32),
            np.zeros((1, r), np.float32),             # per-row key bias
        ], 0).astype(np.float32)
        m = {
            "featsr": featsr.astype(kdt),
            "featslr": featslr.astype(kdt),
            "feats16": f.astype(np.float16),
            "featsl16": fl.astype(np.float16),
            "w2t16": w2t16, "wdt16": wdt16, "bias": bias,
            "ident": ident,
        }
        if split16:
            fr_hi = featsr.astype(np.float16)
            fr_lo = (featsr - fr_hi.astype(np.float32)).astype(np.float16)
            fr_lo[CIN + 1] = 0.0                      # ones row exact in hi
            fl_hi = featslr.astype(np.float16)
            fl_lo = (featslr - fl_hi.astype(np.float32)).astype(np.float16)
            fl_lo[CIN:] = 0.0                         # ones/bias rows in hi
            m.update({"feats_hi": fr_hi, "feats_lo": fr_lo,
                      "featsl_hi": fl_hi, "featsl_lo": fl_lo})
        in_maps.append(m)
    return in_maps


def run_cores(xb, W, b, n, r, trace=False):
    from concourse.bass_utils import run_bass_kernel_spmd

    num = N_CORES
    in_maps = make_in_maps(xb, W, b, n, r, num)
    nc = _get_program(n, r, num)
    res = run_bass_kernel_spmd(nc, in_maps, core_ids=list(range(num)),
                               trace=trace)
    return [res.results[i]["out"] for i in range(num)], res


def kernel(x, W, b):
    """Full-input entry point: x [4, 64, 8192, 1] f32 -> [4, 128, 8192, 1]."""
    x = np.asarray(x, dtype=np.float32)
    W = np.asarray(W, dtype=np.float32)
    b = np.asarray(b, dtype=np.float32)
    xb = np.ascontiguousarray(x[:, :, :, 0])
    r = N // 2
    outs, _ = run_cores(xb, W, b, N, r)
    out = np.empty((B, COUT, N, 1), np.float32)
    for core in range(N_CORES):
        bi, half = core // 2, core % 2
        out[bi, :, half * r:(half + 1) * r, 0] = outs[core]
    return out
